# revision 65
# baseline (speedup 1.0000x reference)
"""ArcFace loss kernel for 8 TRN2 NeuronCores.

Strategy: tensor-parallel over classes (C=50000 -> 6250/core).  Each core
computes cos(emb, w_shard) with an fp8-e4m3 DoubleRow matmul (K=256 per
instruction) and a fused exp+row-sum epilogue on the scalar engine (per-row
1/||e|| folded into the activation scale).  Weight columns stream from HBM
in 1024-class quanta that are squared/normalized/cast to fp8 on the fly, so
the prep pipeline hides under the main matmul+exp loop (group-outer order:
class-group g only needs its own two quanta).  Row/label norms and the
label logit come from fp8 Gram-diagonal matmuls on the transposed operands
(cosine is scale-invariant, so fp8 scale factors self-cancel); the label
path is interleaved into the ragged tail group.  A single AllReduce
combines per-core sum-exp vectors; the margin-corrected log-softmax NLL
mean is computed redundantly on every core.

fp8 scaling: emb is cast raw (components ~N(0,1)); w columns are
normalized on-device and scaled x16 (the 1/16 folds into the exp scale);
wlab is scaled x64 (cancels in the cosine); squares for column norms are
scaled x65536 = 256^2.  The per-class 1/||w|| uses the fast inverse-sqrt
bit trick on the vector engine (bias-corrected magic, ~3% sawtooth that
averages out over 50k classes) so the scalar engine stays dedicated to
the exp+row-sum epilogue, which is the main-loop bottleneck.

Host marshaling: inputs are pre-transposed, k-tile-interleaved per
partition row, and down-cast (embT/wlabT to fp8e4 -- the kernel's first
op was exactly that cast; weights to bf16, which the norm computation
needs) so every DMA moves wide contiguous rows and input bytes shrink
2-4x (DMA throughput here is row-size- and HBM-contention-bound).
Weight pair 1 streams behind the s30 chain and pairs 2/3 behind pool
backpressure so embT + pair0 own the early HBM bandwidth.
"""

import numpy as np

from concourse import bacc, bass, mybir, tile
from concourse import bass_utils
from concourse.bass_interp import get_hw_module
from concourse.masks import make_identity

B, D, C = 2048, 512, 50000
NCORES = 8
CS = C // NCORES            # 6250 classes per core
MARGIN = 0.3
SCALE = 30.0
EPS = 1e-12

F32 = mybir.dt.float32
BF16 = mybir.dt.bfloat16
FP8 = mybir.dt.float8e4
Act = mybir.ActivationFunctionType
Alu = mybir.AluOpType
DR = mybir.MatmulPerfMode.DoubleRow

NB = B // 128               # 16 batch tiles
NK = D // 128               # 4 contraction k-tiles (DR consumes pairs)
W8 = 16.0                   # fp8 scale on normalized weights
WL8 = 64.0                  # fp8 scale on label weights
SQ8 = 65536.0               # scale on squared raw weights (=256^2)

# weight streaming quanta (1024 classes each, ragged 106 tail)
QOFF = [(q * 1024, 1024) for q in range(6)] + [(6144, 106)]
NQ = len(QOFF)              # 7
# weight DMA pairs (two quanta per transfer; host lays these out pair-major)
WPAIRS = [(0, 2048), (2048, 2048), (4096, 2048), (6144, 106)]
# main-loop column groups: one group = two quanta (tail group = q6)
GOFF = [(0, 2048), (2048, 2048), (4096, 2048), (6144, 106)]
NG = len(GOFF)              # 4


def _patch_act_tables():
    """Prefer natural_log_exp_and_others so alternating Ln/Exp activations
    resolve to one table set (avoids ~1.3us ACT_TABLE_LOAD thrash per switch)."""
    import concourse.hw_specs as hw_specs
    import concourse.bacc as bacc_mod
    orig = hw_specs.get_activation_tables
    def filtered(module_arch):
        tables = orig(module_arch)
        pref = "natural_log_exp_and_others"
        if pref in tables:
            tables = {
                k: (v if k == pref else {f for f in v
                                         if f not in tables[pref]})
                for k, v in tables.items()
            }
        return tables
    hw_specs.get_activation_tables = filtered
    bacc_mod.get_activation_tables = filtered


_patch_act_tables()


def build():
    nc = bacc.Bacc("TRN2", debug=False, num_devices=NCORES)

    # host marshals with the D/k dimension interleaved per partition row
    # ([128, NK*cols]) so every DMA moves wide contiguous rows.  embT/wlabT
    # arrive as fp8e4 (the kernel's first op was exactly that cast; values
    # are far from the TRN-vs-OCP e4m3 divergence at 240+); weights arrive
    # bf16 since the norm computation needs the extra mantissa.
    embT_d = nc.dram_tensor("embT", [128, NK * B], FP8, kind="ExternalInput")
    wlabT_d = nc.dram_tensor("wlabT", [128, NK * B], FP8, kind="ExternalInput")
    wT_d = nc.dram_tensor("wT", [128, NK * CS], BF16, kind="ExternalInput")
    out_d = nc.dram_tensor("out", [1, 1], F32, kind="ExternalOutput")

    with tile.TileContext(nc) as tc:
        with (
            tc.tile_pool(name="const", bufs=1) as constp,
            tc.tile_pool(name="res", bufs=1) as resp,
            tc.tile_pool(name="ps", bufs=2, space="PSUM") as psp,
            tc.tile_pool(name="dram", bufs=1, space="DRAM") as dramp,
            tc.tile_pool(name="wraw", bufs=2) as wrawp,
            tc.tile_pool(name="wt28", bufs=3) as wt28p,
            tc.tile_pool(name="nv", bufs=2) as nvbp,
            tc.tile_pool(name="expo", bufs=3) as expop,
            tc.tile_pool(name="gsc", bufs=4) as gscp,
        ):
            # ---- constants ----
            ones8 = constp.tile([128, 2, 128], FP8, tag="ones8")
            nc.vector.memset(ones8[:], 1.0)
            ones_col = constp.tile([128, 1], F32, tag="ones_col")
            nc.vector.memset(ones_col[:], 1.0)
            ident = constp.tile([128, 128], F32, tag="ident")
            make_identity(nc, ident[:])

            # ---- residents ----
            ebT8 = resp.tile([128, NK, B], FP8, tag="ebT8")
            wlT8 = resp.tile([128, NK, B], FP8, tag="wlT8")
            wtn8 = [resp.tile([128, NK, QOFF[q][1]], FP8, tag=f"wtn8_{q}",
                              name=f"wtn8_{q}")
                    for q in range(NQ)]
            Pcols = resp.tile([128, NB * NG], F32, tag="Pcols")
            sse = resp.tile([128, NB], F32, tag="sse")
            inve = resp.tile([128, NB], F32, tag="inve")
            s30 = resp.tile([128, NB], F32, tag="s30")
            ssw = resp.tile([128, NB], F32, tag="ssw")
            dotc = resp.tile([128, NB], F32, tag="dotc")
            cosl = resp.tile([128, NB], F32, tag="cosl")
            invwl = resp.tile([128, NB], F32, tag="invwl")
            corr = resp.tile([128, NB], F32, tag="corr")
            tgt = resp.tile([128, NB], F32, tag="tgt")

            # ---- warm-up collective: stage ncfw/SPAD before the real AR ----
            warm_in = dramp.tile([128, 1], F32, name="warm_in")
            warm_out = dramp.tile([128, 1], F32, name="warm_out",
                                  addr_space="Shared")
            nc.gpsimd.dma_start(warm_in[:], ones_col[:])
            nc.gpsimd.collective_compute(
                "AllReduce", Alu.add, replica_groups=[list(range(NCORES))],
                ins=[warm_in[:].opt()], outs=[warm_out[:].opt()])

            # ---- input DMA issuance ----
            # three parallel queues: scalar HW queue: embT (gates everything,
            # must land first); sync HW queue: weight quanta in need order;
            # gpsimd SW queue (slow but idle): wlabT, needed only late.
            # embT: one flat fp8 DMA straight into the resident tile, on the
            # scalar HW queue so it streams in PARALLEL with pair0 on the
            # sync queue (same queue would serialize them and delay the
            # squares chain that pair0 gates).
            nc.scalar.dma_start(ebT8[:], embT_d.ap()[:, :])

            # weight pairs: host lays pairs out pair-major so each pair is
            # one flat contiguous DMA (128 x 16KB rows); wq[q] -> (tile, base).
            # Only pair0 streams alongside embT (the ramp gate); pair1 is
            # issued from the scalar engine between the casts, pairs 2/3 wait
            # on the bufs=2 pool slots -- so early HBM bandwidth goes to
            # embT + pair0.
            wq = {}
            def issue_wpair(p, eng):
                off, sz = WPAIRS[p]
                t = wrawp.tile([128, NK, 2048], BF16, tag="wraw",
                               name=f"wraw{p}")
                eng.dma_start(
                    t[:, :, :sz], wT_d.ap()[:, NK * off:NK * (off + sz)])
                for j in range((sz + 1023) // 1024):
                    wq[(off + 1024 * j) // 1024] = (t, 1024 * j)

            issue_wpair(0, nc.sync)

            def issue_wlab():
                # issued late in the scalar stream: keeps the scalar HW
                # queue clear for embT early and wlab out of the weight
                # stream's HBM share during the ramp; lands directly in the
                # pre-scaled fp8 resident (host did the x64 + cast)
                nc.scalar.dma_start(wlT8[:], wlabT_d.ap()[:, :])

            # ---- weight quantum prep: squares -> col sum-sq -> nv -> fp8 ----
            def prep_squares(q):
                off, sz = QOFF[q]
                wt28 = wt28p.tile([128, NK, 1024], FP8, tag="wt28")
                wt, base = wq[q]
                for k in range(NK):
                    nc.vector.scalar_tensor_tensor(
                        wt28[:, k, :sz], wt[:, k, base:base + sz],
                        float(SQ8), wt[:, k, base:base + sz],
                        Alu.mult, Alu.mult)
                return wt28

            def prep_ssmm(q, wt28):
                off, sz = QOFF[q]
                ssps = psp.tile([128, 1024], F32, tag="ps")
                for kk in range(NK // 2):
                    for h in range(0, sz, 512):
                        hsz = min(512, sz - h)
                        nc.tensor.matmul(
                            ssps[:, h:h + hsz], ones8[:, :, :128],
                            wt28[:, 2 * kk:2 * kk + 2, h:h + hsz],
                            start=(kk == 0), stop=(kk == NK // 2 - 1),
                            perf_mode=DR)
                return ssps

            def prep_quantum(q):
                return prep_ssmm(q, prep_squares(q))

            # nv = W8*sqrt(SQ8)/sqrt(ss) via the fast inverse-sqrt bit trick
            # on the vector engine (keeps the scalar engine free for exps).
            # Magic folds the 2^12 = W8*sqrt(SQ8) scale and a -0.025 log2
            # bias correction that zeroes the sawtooth's mean (loss rel err
            # ~2e-4 measured vs the exact chain).
            QMAGIC = 0x653426ac

            def chain_and_scale(q, ssps):
                off, sz = QOFF[q]
                nv = nvbp.tile([128, 1024], F32, tag="nv")
                nvi = nv[:, :sz].bitcast(mybir.dt.int32)
                nc.vector.tensor_scalar(
                    nvi, ssps[:, :sz].bitcast(mybir.dt.int32), 1, None,
                    Alu.logical_shift_right)
                nc.vector.tensor_scalar(
                    nvi, nvi, -1, QMAGIC, Alu.mult, Alu.add)
                wt, base = wq[q]
                for k in range(NK):
                    nc.vector.tensor_mul(
                        wtn8[q][:, k, :sz], wt[:, k, base:base + sz],
                        nv[:, :sz])

            def prep_full(q):
                # chain immediately follows prep so the ss psum slot is
                # consumed promptly (late consumption would stall the psum
                # slot rotation against the main-loop cos tiles)
                chain_and_scale(q, prep_quantum(q))

            # vector queue order: the q0/q1 squares first (gated only by the
            # pair0 DMA), gram STTs behind them (gated by ebT8 + gram MMs) --
            # both land around the same time, so neither blocks the other's
            # start.  PSUM allocation order (grams -> ss0 -> ss1 -> cos) is
            # unchanged.
            wt28_0 = prep_squares(0)
            wt28_1 = prep_squares(1)

            # ---- per-batch-tile row norms via fp8 Gram diagonal ----
            for i in range(NB):
                gps = psp.tile([128, 128], F32, tag="ps")
                for kk in range(NK // 2):
                    nc.tensor.matmul(
                        gps[:], ebT8[:, 2 * kk:2 * kk + 2, 128 * i:128 * (i + 1)],
                        ebT8[:, 2 * kk:2 * kk + 2, 128 * i:128 * (i + 1)],
                        start=(kk == 0), stop=(kk == NK // 2 - 1), perf_mode=DR)
                g = gscp.tile([128, 128], F32, tag="gsc")
                nc.vector.scalar_tensor_tensor(
                    g[:], gps[:], 1.0, ident[:], Alu.mult, Alu.mult,
                    accum_out=sse[:, i:i + 1])

            ss0 = prep_ssmm(0, wt28_0)
            chain_and_scale(0, ss0)
            ss1 = prep_ssmm(1, wt28_1)
            chain_and_scale(1, ss1)

            # inve = 1/||e8||; s30 = (SCALE/W8)/||e8||
            nc.scalar.activation(inve[:], sse[:], Act.Ln)
            nc.scalar.activation(inve[:], inve[:], Act.Exp, scale=-0.5)
            nc.vector.tensor_scalar(s30[:], inve[:], float(SCALE / W8),
                                    None, Alu.mult)
            # pair1 deferred behind the s30 chain on the scalar queue so only
            # embT + pair0 compete for HBM during the ramp; pairs 2/3 further
            # wait on the bufs=2 pool slots
            issue_wpair(1, nc.scalar)
            issue_wpair(2, nc.sync)
            issue_wpair(3, nc.sync)

            # ---- label-path helpers (emitted interleaved with tail group) ----
            def label_gram(i):
                gps2 = psp.tile([128, 256], F32, tag="ps")
                for kk in range(NK // 2):
                    nc.tensor.matmul(
                        gps2[:, 0:128],
                        wlT8[:, 2 * kk:2 * kk + 2, 128 * i:128 * (i + 1)],
                        wlT8[:, 2 * kk:2 * kk + 2, 128 * i:128 * (i + 1)],
                        start=(kk == 0), stop=(kk == NK // 2 - 1), perf_mode=DR)
                for kk in range(NK // 2):
                    nc.tensor.matmul(
                        gps2[:, 128:256],
                        ebT8[:, 2 * kk:2 * kk + 2, 128 * i:128 * (i + 1)],
                        wlT8[:, 2 * kk:2 * kk + 2, 128 * i:128 * (i + 1)],
                        start=(kk == 0), stop=(kk == NK // 2 - 1), perf_mode=DR)
                g2 = gscp.tile([128, 128], F32, tag="gsc")
                nc.vector.scalar_tensor_tensor(
                    g2[:], gps2[:, 0:128], 1.0, ident[:], Alu.mult, Alu.mult,
                    accum_out=ssw[:, i:i + 1])
                g3 = gscp.tile([128, 128], F32, tag="gsc")
                nc.vector.scalar_tensor_tensor(
                    g3[:], gps2[:, 128:256], 1.0, ident[:], Alu.mult, Alu.mult,
                    accum_out=dotc[:, i:i + 1])

            def label_finish():
                # invwl = 1/||wl8|| (x64 scale cancels in cosl)
                nc.scalar.activation(invwl[:], ssw[:], Act.Ln)
                nc.scalar.activation(invwl[:], invwl[:], Act.Exp, scale=-0.5)
                nc.vector.tensor_mul(cosl[:], dotc[:], inve[:])
                nc.vector.tensor_mul(cosl[:], cosl[:], invwl[:])
                # corr = exp(30*cosl) * (exp(-9) - 1); tgt = 30*cosl - 9
                e1 = gscp.tile([128, 128], F32, tag="gsc")
                nc.scalar.activation(e1[:, 0:NB], cosl[:], Act.Exp,
                                     bias=0.0, scale=float(SCALE))
                nc.vector.tensor_scalar(
                    corr[:], e1[:, 0:NB], float(np.exp(-MARGIN * SCALE) - 1.0),
                    None, Alu.mult)
                nc.vector.tensor_scalar(
                    tgt[:], cosl[:], float(SCALE), float(-MARGIN * SCALE),
                    Alu.mult, Alu.add)

            # ---- main loop: group-outer, batch-tile-inner ----
            # interleave map: (g, i) -> thunks emitted after that step (weight
            # quanta for group g+1 prep while group g computes)
            # squares are emitted two iterations before the ss-matmul+chain
            # so the tensor-queue ss MM never waits on the vector squares
            # (that wait was stalling the next cos matmuls behind it)
            wt28h = {}
            def sq_at(q):
                wt28h[q] = prep_squares(q)
            def rest_at(q):
                chain_and_scale(q, prep_ssmm(q, wt28h[q]))
            inter = {
                (0, 2): [lambda: sq_at(2)],
                (0, 4): [lambda: rest_at(2)],
                (0, 6): [lambda: sq_at(3)],
                (0, 8): [lambda: rest_at(3)],
                (1, 2): [lambda: sq_at(4), issue_wlab],
                (1, 4): [lambda: rest_at(4)],
                (1, 6): [lambda: sq_at(5)],
                (1, 8): [lambda: rest_at(5)],
                (2, 5): [lambda: sq_at(6)],
                (2, 7): [lambda: rest_at(6)],
            }

            for g in range(NG):
                goff, gsz = GOFF[g]
                for i in range(NB):
                    ps = psp.tile([128, 2048], F32, tag="ps",
                                  name=f"cos{g}_{i}")
                    for kk in range(NK // 2):
                        for h in range(0, gsz, 512):
                            hh = min(512, gsz - h)
                            q = (goff + h) // 1024
                            qo = (goff + h) % 1024
                            nc.tensor.matmul(
                                ps[:, h:h + hh],
                                ebT8[:, 2 * kk:2 * kk + 2,
                                     128 * i:128 * (i + 1)],
                                wtn8[q][:, 2 * kk:2 * kk + 2, qo:qo + hh],
                                start=(kk == 0), stop=(kk == NK // 2 - 1),
                                perf_mode=DR)
                    ex = expop.tile([128, 2048], BF16, tag="ex",
                                    name=f"ex{g}_{i}")
                    nc.scalar.activation(
                        ex[:, :gsz], ps[:, :gsz], Act.Exp,
                        bias=0.0, scale=s30[:, i:i + 1],
                        accum_out=Pcols[:, i * NG + g:i * NG + g + 1])
                    if g == 3 and i < 8:
                        # tail group has tensor slack: slip in label grams
                        label_gram(2 * i)
                        label_gram(2 * i + 1)
                    if g == 3 and i == 8:
                        # all label grams emitted; get corr/tgt off the tail
                        label_finish()
                    for fn in inter.get((g, i), []):
                        fn()

            # ---- AllReduce + loss ----
            with tc.tile_pool(name="fin", bufs=1) as finp:
                P = finp.tile([128, NB], F32, tag="P")
                nc.vector.tensor_reduce(
                    P[:], Pcols[:].rearrange("p (i j) -> p i j", j=NG),
                    mybir.AxisListType.X, Alu.add)
                cc_in = dramp.tile([128, NB], F32, name="cc_in")
                cc_out = dramp.tile([128, NB], F32, name="cc_out",
                                    addr_space="Shared")
                nc.scalar.dma_start(cc_in[:], P[:])
                nc.gpsimd.collective_compute(
                    "AllReduce", Alu.add,
                    replica_groups=[list(range(NCORES))],
                    ins=[cc_in[:].opt()], outs=[cc_out[:].opt()])
                Ssb = finp.tile([128, NB], F32, tag="Ssb")
                nc.scalar.dma_start(Ssb[:], cc_out[:])
                S = finp.tile([128, NB], F32, tag="S")
                nc.vector.tensor_add(S[:], Ssb[:], corr[:])
                lnS = finp.tile([128, NB], F32, tag="lnS")
                nc.scalar.activation(lnS[:], S[:], Act.Ln)
                nll = finp.tile([128, NB], F32, tag="nll")
                nc.vector.tensor_sub(nll[:], lnS[:], tgt[:])
                nrow = finp.tile([128, 1], F32, tag="nrow")
                nc.vector.tensor_reduce(
                    nrow[:], nll[:], mybir.AxisListType.X, Alu.add)
                ps11 = psp.tile([1, 1], F32, tag="ps", padded_shape=[1, 128])
                nc.tensor.matmul(ps11[:], ones_col[:], nrow[:],
                                 start=True, stop=True)
                loss_sb = finp.tile([1, 1], F32, tag="loss_sb")
                nc.scalar.mul(loss_sb[:], ps11[:], 1.0 / B)
                nc.sync.dma_start(out_d.ap()[:, :], loss_sb[:])

    nc.compile()
    nc.m = get_hw_module(nc.m)
    return nc


_NC_CACHE = None


def _get_nc():
    global _NC_CACHE
    if _NC_CACHE is None:
        _NC_CACHE = build()
    return _NC_CACHE


def _interleave_k(mat_T, dt):
    """[D, N] -> [128, NK*N] in dtype dt: partition row p holds k-tile rows
    p, 128+p, 256+p, 384+p concatenated (k-major along the free dim)."""
    D_, N = mat_T.shape
    out = mat_T.reshape(NK, 128, N).transpose(1, 0, 2).reshape(128, NK * N)
    return np.ascontiguousarray(out.astype(dt))


def make_in_maps(embeddings, labels, weight):
    import ml_dtypes
    bf16 = ml_dtypes.bfloat16
    f8 = ml_dtypes.float8_e4m3fn  # == TRN fp8e4 bit layout for |x| <= 240
    embeddings = np.asarray(embeddings, dtype=np.float32)
    weight = np.asarray(weight, dtype=np.float32)
    labels_i = np.asarray(labels).astype(np.int64)

    embT = _interleave_k(embeddings.T, f8)
    wlabT = _interleave_k(weight[labels_i].T * WL8, f8)

    in_maps = []
    for c in range(NCORES):
        shard = weight[c * CS:(c + 1) * CS]               # [6250, 512]
        shardT = np.ascontiguousarray(shard.T)            # [512, 6250]
        blocks = [_interleave_k(shardT[:, off:off + sz], bf16)
                  for off, sz in WPAIRS]
        wT = np.ascontiguousarray(np.concatenate(blocks, axis=1))
        in_maps.append({"embT": embT, "wlabT": wlabT, "wT": wT})
    return in_maps


def kernel(embeddings, labels, weight, _trace=False, _trace_kwargs=None):
    in_maps = make_in_maps(embeddings, labels, weight)
    nc = _get_nc()
    res = bass_utils.run_bass_kernel_spmd(
        nc, in_maps, core_ids=list(range(NCORES)),
        trace=_trace, **(_trace_kwargs or {}))
    out = np.asarray(res.results[0]["out"], dtype=np.float32).reshape(())
    if _trace:
        kernel.last_result = res
    return out
